# revision 15
# baseline (speedup 1.0000x reference)
"""HMM forward-algorithm (MsaHmmCell) Trainium2 kernel.

Full inputs in, full outputs out. Internally: data-parallel over batch across
8 NeuronCores (32 sequences per core); each core holds the full transition /
emission matrices in SBUF and runs the T=512-step sequential scan.

Device-side formulation (per core):
  - state kept TRANSPOSED: s_t stored (128 part = q mod 128, 9*32 free) where
    free index j = 32*c + b encodes q-chunk c (q = 128*c + p) and batch b.
  - transition: s@A as 81 accumulating matmuls, out = lhsT.T @ rhs with
    lhsT = A-tile (k-chunk part, m-chunk free) resident bf16 (FWL), rhs =
    s-chunk (128, 32). Output stays transposed -> no per-step transposes.
  - emission E_t = B.T[x_t] via one-hot matmul (BT-chunk lhsT, onehot rhs).
  - normalization is NOT done per step on device. Instead every step is
    rescaled by r = 1/sigma from TWO steps earlier (folded into the emission
    scale via a K=1 broadcast matmul), which keeps magnitudes bounded while
    keeping the reciprocal off the PE critical path. Stored per-step column
    sums sigma and the applied r let the host reconstruct the exactly
    normalized forward variables and the log-likelihood.
"""

import os

import numpy as np
import ml_dtypes

import concourse.bass as bass
import concourse.mybir as mybir
import concourse.tile as tile
from concourse.bass_utils import run_bass_kernel_spmd

Q = 1027
ALPHA = 26
BATCH = 256
T = 512
NCORES = 8
BC = BATCH // NCORES          # 32 sequences per core
NCH = 9                       # q chunks
QP = NCH * 128                # padded state count 1152
SW = NCH * BC                 # state free width 288

BF16 = mybir.dt.bfloat16
F32 = mybir.dt.float32

LAST_RESULTS = None           # BassKernelResults of the most recent run (for test.py)
TRACE = bool(os.environ.get("HMM_TRACE"))


def _split_drain_waits(nc, max_waits=1):
    """This walrus build encodes at most one sync wait per instruction.
    Move excess waits onto preceding single-wait NoOps (same engine, in-order
    execution => semantically identical)."""
    for f in nc.m.functions:
        for bb in f.blocks:
            new_insts = []
            for inst in bb.instructions:
                si = getattr(inst, "sync_info", None)
                if (
                    si is not None
                    and si.on_wait
                    and len(si.on_wait) > max_waits
                ):
                    waits = list(si.on_wait)
                    excess, keep = waits[:-max_waits], waits[-max_waits:]
                    for w in excess:
                        new_insts.append(
                            mybir.InstNoOp(
                                name=f"I-{nc.next_id()}",
                                engine=inst.engine,
                                ins=[],
                                outs=[],
                                sync_info=mybir.SyncInfo(on_wait=[w], on_update=[]),
                            )
                        )
                    inst.sync_info = mybir.SyncInfo(
                        on_wait=keep, on_update=list(si.on_update or [])
                    )
                new_insts.append(inst)
            bb.instructions[:] = new_insts


def _build(nsteps=T):
    nc = bass.Bass("TRN2", target_bir_lowering=False, debug=False)

    a_in = nc.dram_tensor("a_t", (128, NCH * NCH * 128), BF16, kind="ExternalInput")
    e_in = nc.dram_tensor("e_t", (nsteps, 128, SW), BF16, kind="ExternalInput")
    init_in = nc.dram_tensor("init_t", (128, NCH), F32, kind="ExternalInput")

    s_out = nc.dram_tensor("s_out", (nsteps, 128, SW), BF16, kind="ExternalOutput")
    m_out = nc.dram_tensor("m_out", (1, nsteps * BC), F32, kind="ExternalOutput")
    r_out = nc.dram_tensor("r_out", (1, nsteps * BC), BF16, kind="ExternalOutput")

    KA = 2          # k-chunks covered by the early state tile
    CH0 = KA * BC   # 64 columns

    with tile.TileContext(nc) as tc:
        with (
            tc.tile_pool(name="const", bufs=1) as const,
            tc.tile_pool(name="state", bufs=4) as state,
            tc.tile_pool(name="emis", bufs=4) as emis,
            tc.tile_pool(name="small", bufs=3) as small,
            tc.tile_pool(name="ph", bufs=3, space="PSUM") as ph_pool,
            tc.tile_pool(name="pbc", bufs=2, space="PSUM") as pbc_pool,
            tc.tile_pool(name="psig", bufs=2, space="PSUM") as psig_pool,
        ):
            # ---- resident constants ----
            a_sb = const.tile([128, NCH * NCH * 128], BF16)
            nc.gpsimd.dma_start(out=a_sb, in_=a_in[:, :])
            init_sb = const.tile([128, NCH], F32)
            nc.gpsimd.dma_start(out=init_sb, in_=init_in[:, :])
            ones_k = const.tile([128, 1], BF16)
            nc.vector.memset(ones_k, 1.0)
            ones_m = const.tile([1, 128], BF16)
            nc.vector.memset(ones_m, 1.0)
            # per-step scalars accumulate on-chip; single DMA at the end
            # (tiny per-step DMAs clog the HWDGE queues)
            m_acc = const.tile([1, nsteps * BC], F32)
            r_acc = const.tile([1, nsteps * BC], BF16)

            def eload(t):
                E = emis.tile([128, SW], BF16, tag="E")
                nc.gpsimd.dma_start(out=E, in_=e_in[t])
                return E

            def schunk(s, k):
                """slice of the (sA, sB) state pair covering k-chunk k"""
                sa, sb = s
                if k < KA:
                    return sa[:, k * BC : (k + 1) * BC]
                return sb[:, (k - KA) * BC : (k - KA + 1) * BC]

            def sigma(t, s):
                """column sums -> psum (1, BC); write m_out[t]"""
                psig = psig_pool.tile([1, BC], F32, tag="psig")
                for c in range(NCH):
                    nc.tensor.matmul(
                        psig,
                        ones_k,
                        schunk(s, c),
                        start=(c == 0),
                        stop=(c == NCH - 1),
                    )
                nc.scalar.copy(out=m_acc[:, t * BC : (t + 1) * BC], in_=psig)
                return psig

            def recip(t, psig):
                # bf16 reciprocal: the value applied on-device IS the stored
                # value the host uses, so its rounding cancels exactly.
                rb = r_acc[:, t * BC : (t + 1) * BC]
                with nc.allow_low_precision("r rounding tracked exactly on host"):
                    nc.vector.reciprocal(out=rb, in_=psig)
                return rb

            def bcast(r):
                """broadcast r to all partitions/chunks as bf16 in SBUF"""
                pbc = pbc_pool.tile([128, SW], F32, tag="pbc")
                for c in range(NCH):
                    nc.tensor.matmul(
                        pbc[:, c * BC : (c + 1) * BC], ones_m, r, start=True, stop=True
                    )
                rbc = emis.tile([128, SW], BF16, tag="rbc")
                nc.scalar.copy(out=rbc, in_=pbc)
                return rbc

            def state_mul(ph, esc):
                """s = ph * esc as (sA, sB) pair; sA small and first so
                next-step k<KA matmuls release early"""
                sa = state.tile([128, CH0], BF16, tag="sa")
                sb = state.tile([128, SW - CH0], BF16, tag="sb")
                nc.vector.tensor_mul(out=sa, in0=ph[:, :CH0], in1=esc[:, :CH0])
                nc.vector.tensor_mul(out=sb, in0=ph[:, CH0:], in1=esc[:, CH0:])
                return sa, sb

            # ---- t = 0 ----
            E0 = eload(0)
            s0a = state.tile([128, CH0], BF16, tag="sa")
            s0b = state.tile([128, SW - CH0], BF16, tag="sb")
            for c in range(NCH):
                nc.vector.tensor_scalar_mul(
                    out=schunk((s0a, s0b), c),
                    in0=E0[:, c * BC : (c + 1) * BC],
                    scalar1=init_sb[:, c : c + 1],
                )
            s_prev = (s0a, s0b)
            nc.sync.dma_start(out=s_out[0][:, :CH0], in_=s0a)
            nc.sync.dma_start(out=s_out[0][:, CH0:], in_=s0b)
            psig = sigma(0, s_prev)
            r_prev = recip(0, psig)       # most recent computed r
            rbc = bcast(r_prev)
            E1 = eload(1)
            esc_prev = emis.tile([128, SW], BF16, tag="esc")
            nc.vector.tensor_mul(out=esc_prev, in0=E1, in1=rbc)
            sig_pending = None            # (t, state) whose sigma is deferred

            # ---- t = 1 .. nsteps-1 ----
            # iteration t: trans(t) + state-mul(t); sigma/recip for step t-1
            # (deferred so those matmuls never wait on the fresh state);
            # Esc(t+1) scaled with the newest r available without waiting.
            for t in range(1, nsteps):
                # deferred sigma/recip for step t-1: the sigma matmuls read a
                # state that is already complete (no PE wait), and the DVE
                # reciprocal overlaps the transition block below.
                if sig_pending is not None:
                    pt, ps = sig_pending
                    psig = sigma(pt, ps)
                    r_prev = recip(pt, psig)
                    sig_pending = None
                ph = ph_pool.tile([128, SW], F32, tag="ph")
                for k in range(NCH):
                    for m in range(NCH):
                        nc.tensor.matmul(
                            ph[:, m * BC : (m + 1) * BC],
                            a_sb[:, (k * NCH + m) * 128 : (k * NCH + m + 1) * 128],
                            schunk(s_prev, k),
                            # start clears has_written for the WHOLE bank: only
                            # the first MM of the step may set it.
                            start=(k == 0 and m == 0),
                            stop=(k == NCH - 1 and m == NCH - 1),
                            skip_group_check=True,
                        )
                s_cur = state_mul(ph, esc_prev)
                nc.sync.dma_start(out=s_out[t][:, :CH0], in_=s_cur[0])
                nc.sync.dma_start(out=s_out[t][:, CH0:], in_=s_cur[1])

                # Esc(t+1) scaled by r_{t-1} (computed above, overlapped with
                # the transition block): stable lag-2 rescale feedback.
                if t < nsteps - 1:
                    E_next = eload(t + 1)
                    rbc = bcast(r_prev)
                    esc_prev = emis.tile([128, SW], BF16, tag="esc")
                    nc.vector.tensor_mul(out=esc_prev, in0=E_next, in1=rbc)
                sig_pending = (t, s_cur)
                s_prev = s_cur

            # final step's sigma (no recip needed)
            pt, ps = sig_pending
            sigma(pt, ps)
            nc.sync.dma_start(out=m_out[:, :], in_=m_acc)
            nc.sync.dma_start(out=r_out[:, :], in_=r_acc)

    _split_drain_waits(nc)
    return nc


def _softmax(x, axis=-1):
    x = np.asarray(x, dtype=np.float32)
    m = x.max(axis=axis, keepdims=True)
    e = np.exp(x - m, dtype=np.float32)
    return e / e.sum(axis=axis, keepdims=True, dtype=np.float32)


_CACHE = {}


def kernel(x, A_logits, B_logits, init_logits):
    global LAST_RESULTS
    x = np.asarray(x, dtype=np.int32)
    A_logits = np.asarray(A_logits, dtype=np.float32)
    B_logits = np.asarray(B_logits, dtype=np.float32)
    init_logits = np.asarray(init_logits, dtype=np.float32)

    # ---- host prep ----
    A = _softmax(A_logits)                       # (Q, Q)
    Bm = _softmax(B_logits)                      # (Q, ALPHA)
    init = _softmax(init_logits)                 # (Q,)

    A_pad = np.zeros((QP, QP), np.float32)
    A_pad[:Q, :Q] = A
    # a_host[p, k, m, c] = A_pad[k*128+p, m*128+c]
    a_host = np.ascontiguousarray(
        A_pad.reshape(NCH, 128, NCH, 128).transpose(1, 0, 2, 3)
    ).reshape(128, NCH * NCH * 128).astype(ml_dtypes.bfloat16)

    Bpad = np.zeros((QP, ALPHA), np.float32)
    Bpad[:Q] = Bm

    init_pad = np.zeros((QP,), np.float32)
    init_pad[:Q] = init
    init_host = np.ascontiguousarray(init_pad.reshape(NCH, 128).T)  # (128, NCH)

    in_maps = []
    for i in range(NCORES):
        xi = x[i * BC : (i + 1) * BC]            # (BC, T)
        # E_host[t, p, c*BC+b] = Bpad[128c+p, x[b,t]]
        e = Bpad[:, xi]                          # (QP, BC, T)
        e = e.reshape(NCH, 128, BC, T).transpose(3, 1, 0, 2).reshape(T, 128, SW)
        in_maps.append(
            {
                "a_t": a_host,
                "e_t": np.ascontiguousarray(e).astype(ml_dtypes.bfloat16),
                "init_t": init_host,
            }
        )

    if "nc" not in _CACHE:
        _CACHE["nc"] = _build(T)
    nc = _CACHE["nc"]

    res = run_bass_kernel_spmd(
        nc, in_maps, core_ids=list(range(NCORES)), trace=TRACE
    )
    LAST_RESULTS = res

    # ---- host reconstruction ----
    forward = np.empty((BATCH, T, Q), np.float32)
    loglik = np.empty((BATCH, 1), np.float32)
    # index of the r applied at step t (folded into E_t): r_{max(t-2, 0)}
    ridx = np.maximum(np.arange(1, T) - 2, 0)
    for i in range(NCORES):
        out = res.results[i]
        s = out["s_out"].astype(np.float32)      # (T, 128, SW)
        mm = out["m_out"].reshape(T, BC).astype(np.float64)
        rr = out["r_out"].reshape(T, BC).astype(np.float64)
        # forward[b, t, q=c*128+p] = s[t, p, c*32+b] / m[t, b]
        sr = s.reshape(T, 128, NCH, BC).transpose(3, 0, 2, 1).reshape(BC, T, QP)
        fw = sr[:, :, :Q] / mm.T[:, :, None].astype(np.float32)
        forward[i * BC : (i + 1) * BC] = fw
        # log S_t: S_0 = m_0; S_t = m_t / (r_{ridx(t)} * m_{t-1})
        logm = np.log(mm)                        # (T, BC)
        logr = np.log(np.where(rr > 0, rr, 1.0))  # (T, BC); last rows unused
        ll = logm[0] + np.sum(logm[1:] - logm[:-1] - logr[ridx], axis=0)
        loglik[i * BC : (i + 1) * BC, 0] = ll.astype(np.float32)

    return forward, loglik


# revision 16
# speedup vs baseline: 1.0294x; 1.0294x over previous
"""HMM forward-algorithm (MsaHmmCell) Trainium2 kernel.

Full inputs in, full outputs out. Internally: data-parallel over batch across
8 NeuronCores (32 sequences per core); each core holds the full transition /
emission matrices in SBUF and runs the T=512-step sequential scan.

Device-side formulation (per core):
  - state kept TRANSPOSED: s_t stored (128 part = q mod 128, 9*32 free) where
    free index j = 32*c + b encodes q-chunk c (q = 128*c + p) and batch b.
  - transition: s@A as 81 accumulating matmuls, out = lhsT.T @ rhs with
    lhsT = A-tile (k-chunk part, m-chunk free) resident bf16 (FWL), rhs =
    s-chunk (128, 32). Output stays transposed -> no per-step transposes.
  - emission E_t = B.T[x_t] via one-hot matmul (BT-chunk lhsT, onehot rhs).
  - normalization is NOT done per step on device. Instead every step is
    rescaled by r = 1/sigma from TWO steps earlier (folded into the emission
    scale via a K=1 broadcast matmul), which keeps magnitudes bounded while
    keeping the reciprocal off the PE critical path. Stored per-step column
    sums sigma and the applied r let the host reconstruct the exactly
    normalized forward variables and the log-likelihood.
"""

import os

import numpy as np
import ml_dtypes

import concourse.bass as bass
import concourse.mybir as mybir
import concourse.tile as tile
from concourse.bass_utils import run_bass_kernel_spmd

Q = 1027
ALPHA = 26
BATCH = 256
T = 512
NCORES = 8
BC = BATCH // NCORES          # 32 sequences per core
NCH = 9                       # q chunks
QP = NCH * 128                # padded state count 1152
SW = NCH * BC                 # state free width 288

BF16 = mybir.dt.bfloat16
F32 = mybir.dt.float32

LAST_RESULTS = None           # BassKernelResults of the most recent run (for test.py)
TRACE = bool(os.environ.get("HMM_TRACE"))


def _split_drain_waits(nc, max_waits=1):
    """This walrus build encodes at most one sync wait per instruction.
    Move excess waits onto preceding single-wait NoOps (same engine, in-order
    execution => semantically identical)."""
    for f in nc.m.functions:
        for bb in f.blocks:
            new_insts = []
            for inst in bb.instructions:
                si = getattr(inst, "sync_info", None)
                if (
                    si is not None
                    and si.on_wait
                    and len(si.on_wait) > max_waits
                ):
                    waits = list(si.on_wait)
                    excess, keep = waits[:-max_waits], waits[-max_waits:]
                    for w in excess:
                        new_insts.append(
                            mybir.InstNoOp(
                                name=f"I-{nc.next_id()}",
                                engine=inst.engine,
                                ins=[],
                                outs=[],
                                sync_info=mybir.SyncInfo(on_wait=[w], on_update=[]),
                            )
                        )
                    inst.sync_info = mybir.SyncInfo(
                        on_wait=keep, on_update=list(si.on_update or [])
                    )
                new_insts.append(inst)
            bb.instructions[:] = new_insts


def _build(nsteps=T):
    nc = bass.Bass("TRN2", target_bir_lowering=False, debug=False)

    a_in = nc.dram_tensor("a_t", (128, NCH * NCH * 128), BF16, kind="ExternalInput")
    e_in = nc.dram_tensor("e_t", (nsteps, 128, SW), BF16, kind="ExternalInput")
    init_in = nc.dram_tensor("init_t", (128, NCH), F32, kind="ExternalInput")

    s_out = nc.dram_tensor("s_out", (nsteps, 128, SW), BF16, kind="ExternalOutput")
    m_out = nc.dram_tensor("m_out", (1, nsteps * BC), F32, kind="ExternalOutput")
    r_out = nc.dram_tensor("r_out", (1, nsteps * BC), BF16, kind="ExternalOutput")

    KA = 2          # k-chunks covered by the early state tile
    CH0 = KA * BC   # 64 columns

    with tile.TileContext(nc) as tc:
        with (
            tc.tile_pool(name="const", bufs=1) as const,
            tc.tile_pool(name="state", bufs=4) as state,
            tc.tile_pool(name="emis", bufs=4) as emis,
            tc.tile_pool(name="small", bufs=3) as small,
            tc.tile_pool(name="ph", bufs=3, space="PSUM") as ph_pool,
            tc.tile_pool(name="pbc", bufs=2, space="PSUM") as pbc_pool,
            tc.tile_pool(name="psig", bufs=2, space="PSUM") as psig_pool,
        ):
            # ---- resident constants ----
            a_sb = const.tile([128, NCH * NCH * 128], BF16)
            nc.gpsimd.dma_start(out=a_sb, in_=a_in[:, :])
            init_sb = const.tile([128, NCH], F32)
            nc.gpsimd.dma_start(out=init_sb, in_=init_in[:, :])
            ones_k = const.tile([128, 1], BF16)
            nc.vector.memset(ones_k, 1.0)
            ones_m = const.tile([1, 128], BF16)
            nc.vector.memset(ones_m, 1.0)
            # per-step scalars accumulate on-chip; single DMA at the end
            # (tiny per-step DMAs clog the HWDGE queues)
            m_acc = const.tile([1, nsteps * BC], F32)
            r_acc = const.tile([1, nsteps * BC], BF16)

            def eload(t):
                E = emis.tile([128, SW], BF16, tag="E")
                nc.gpsimd.dma_start(out=E, in_=e_in[t])
                return E

            def schunk(s, k):
                """slice of the (sA, sB) state pair covering k-chunk k"""
                sa, sb = s
                if k < KA:
                    return sa[:, k * BC : (k + 1) * BC]
                return sb[:, (k - KA) * BC : (k - KA + 1) * BC]

            def sigma(t, s):
                """column sums -> psum (1, BC); write m_out[t]"""
                psig = psig_pool.tile([1, BC], F32, tag="psig")
                for c in range(NCH):
                    nc.tensor.matmul(
                        psig,
                        ones_k,
                        schunk(s, c),
                        start=(c == 0),
                        stop=(c == NCH - 1),
                    )
                nc.scalar.copy(out=m_acc[:, t * BC : (t + 1) * BC], in_=psig)
                return psig

            def recip(t, psig):
                # bf16 reciprocal: the value applied on-device IS the stored
                # value the host uses, so its rounding cancels exactly.
                rb = r_acc[:, t * BC : (t + 1) * BC]
                with nc.allow_low_precision("r rounding tracked exactly on host"):
                    nc.vector.reciprocal(out=rb, in_=psig)
                return rb

            def bcast(r):
                """broadcast r to all partitions/chunks as bf16 in SBUF"""
                pbc = pbc_pool.tile([128, SW], F32, tag="pbc")
                for c in range(NCH):
                    nc.tensor.matmul(
                        pbc[:, c * BC : (c + 1) * BC], ones_m, r, start=True, stop=True
                    )
                rbc = emis.tile([128, SW], BF16, tag="rbc")
                nc.scalar.copy(out=rbc, in_=pbc)
                return rbc

            def state_mul(ph, esc):
                """s = ph * esc as (sA, sB) pair; sA small and first so
                next-step k<KA matmuls release early"""
                sa = state.tile([128, CH0], BF16, tag="sa")
                sb = state.tile([128, SW - CH0], BF16, tag="sb")
                nc.vector.tensor_mul(out=sa, in0=ph[:, :CH0], in1=esc[:, :CH0])
                nc.vector.tensor_mul(out=sb, in0=ph[:, CH0:], in1=esc[:, CH0:])
                return sa, sb

            # ---- t = 0 ----
            E0 = eload(0)
            s0a = state.tile([128, CH0], BF16, tag="sa")
            s0b = state.tile([128, SW - CH0], BF16, tag="sb")
            for c in range(NCH):
                nc.vector.tensor_scalar_mul(
                    out=schunk((s0a, s0b), c),
                    in0=E0[:, c * BC : (c + 1) * BC],
                    scalar1=init_sb[:, c : c + 1],
                )
            s_prev = (s0a, s0b)
            nc.sync.dma_start(out=s_out[0][:, :CH0], in_=s0a)
            nc.sync.dma_start(out=s_out[0][:, CH0:], in_=s0b)
            psig = sigma(0, s_prev)
            r_prev = recip(0, psig)       # most recent computed r
            rbc = bcast(r_prev)
            E1 = eload(1)
            esc_prev = emis.tile([128, SW], BF16, tag="esc")
            nc.vector.tensor_mul(out=esc_prev, in0=E1, in1=rbc)
            sig_pending = None            # (t, state) whose sigma is deferred

            # ---- t = 1 .. nsteps-1 ----
            # iteration t: trans(t) + state-mul(t); sigma/recip for step t-1
            # (deferred so those matmuls never wait on the fresh state);
            # Esc(t+1) scaled with the newest r available without waiting.
            for t in range(1, nsteps):
                ph = ph_pool.tile([128, SW], F32, tag="ph")

                def trans(k):
                    for m in range(NCH):
                        nc.tensor.matmul(
                            ph[:, m * BC : (m + 1) * BC],
                            a_sb[:, (k * NCH + m) * 128 : (k * NCH + m + 1) * 128],
                            schunk(s_prev, k),
                            # start clears has_written for the WHOLE bank: only
                            # the first MM of the step may set it.
                            start=(k == 0 and m == 0),
                            stop=(k == NCH - 1 and m == NCH - 1),
                            skip_group_check=True,
                        )

                # k=0,1 need only the small early state tile (sA); the deferred
                # sigma block sits after them so its late chunks (and the DVE
                # reciprocal) overlap the rest of the transition block.
                trans(0)
                trans(1)
                if sig_pending is not None:
                    pt, ps = sig_pending
                    psig = sigma(pt, ps)
                    r_prev = recip(pt, psig)
                    sig_pending = None
                for k in range(2, NCH):
                    trans(k)
                s_cur = state_mul(ph, esc_prev)
                nc.sync.dma_start(out=s_out[t][:, :CH0], in_=s_cur[0])
                nc.sync.dma_start(out=s_out[t][:, CH0:], in_=s_cur[1])

                # Esc(t+1) scaled by r_{t-1} (computed above, overlapped with
                # the transition block): stable lag-2 rescale feedback.
                if t < nsteps - 1:
                    E_next = eload(t + 1)
                    rbc = bcast(r_prev)
                    esc_prev = emis.tile([128, SW], BF16, tag="esc")
                    nc.vector.tensor_mul(out=esc_prev, in0=E_next, in1=rbc)
                sig_pending = (t, s_cur)
                s_prev = s_cur

            # final step's sigma (no recip needed)
            pt, ps = sig_pending
            sigma(pt, ps)
            nc.sync.dma_start(out=m_out[:, :], in_=m_acc)
            nc.sync.dma_start(out=r_out[:, :], in_=r_acc)

    _split_drain_waits(nc)
    return nc


def _softmax(x, axis=-1):
    x = np.asarray(x, dtype=np.float32)
    m = x.max(axis=axis, keepdims=True)
    e = np.exp(x - m, dtype=np.float32)
    return e / e.sum(axis=axis, keepdims=True, dtype=np.float32)


_CACHE = {}


def kernel(x, A_logits, B_logits, init_logits):
    global LAST_RESULTS
    x = np.asarray(x, dtype=np.int32)
    A_logits = np.asarray(A_logits, dtype=np.float32)
    B_logits = np.asarray(B_logits, dtype=np.float32)
    init_logits = np.asarray(init_logits, dtype=np.float32)

    # ---- host prep ----
    A = _softmax(A_logits)                       # (Q, Q)
    Bm = _softmax(B_logits)                      # (Q, ALPHA)
    init = _softmax(init_logits)                 # (Q,)

    A_pad = np.zeros((QP, QP), np.float32)
    A_pad[:Q, :Q] = A
    # a_host[p, k, m, c] = A_pad[k*128+p, m*128+c]
    a_host = np.ascontiguousarray(
        A_pad.reshape(NCH, 128, NCH, 128).transpose(1, 0, 2, 3)
    ).reshape(128, NCH * NCH * 128).astype(ml_dtypes.bfloat16)

    Bpad = np.zeros((QP, ALPHA), np.float32)
    Bpad[:Q] = Bm

    init_pad = np.zeros((QP,), np.float32)
    init_pad[:Q] = init
    init_host = np.ascontiguousarray(init_pad.reshape(NCH, 128).T)  # (128, NCH)

    in_maps = []
    for i in range(NCORES):
        xi = x[i * BC : (i + 1) * BC]            # (BC, T)
        # E_host[t, p, c*BC+b] = Bpad[128c+p, x[b,t]]
        e = Bpad[:, xi]                          # (QP, BC, T)
        e = e.reshape(NCH, 128, BC, T).transpose(3, 1, 0, 2).reshape(T, 128, SW)
        in_maps.append(
            {
                "a_t": a_host,
                "e_t": np.ascontiguousarray(e).astype(ml_dtypes.bfloat16),
                "init_t": init_host,
            }
        )

    if "nc" not in _CACHE:
        _CACHE["nc"] = _build(T)
    nc = _CACHE["nc"]

    res = run_bass_kernel_spmd(
        nc, in_maps, core_ids=list(range(NCORES)), trace=TRACE
    )
    LAST_RESULTS = res

    # ---- host reconstruction ----
    forward = np.empty((BATCH, T, Q), np.float32)
    loglik = np.empty((BATCH, 1), np.float32)
    # index of the r applied at step t (folded into E_t): r_{max(t-2, 0)}
    ridx = np.maximum(np.arange(1, T) - 2, 0)
    for i in range(NCORES):
        out = res.results[i]
        s = out["s_out"].astype(np.float32)      # (T, 128, SW)
        mm = out["m_out"].reshape(T, BC).astype(np.float64)
        rr = out["r_out"].reshape(T, BC).astype(np.float64)
        # forward[b, t, q=c*128+p] = s[t, p, c*32+b] / m[t, b]
        sr = s.reshape(T, 128, NCH, BC).transpose(3, 0, 2, 1).reshape(BC, T, QP)
        fw = sr[:, :, :Q] / mm.T[:, :, None].astype(np.float32)
        forward[i * BC : (i + 1) * BC] = fw
        # log S_t: S_0 = m_0; S_t = m_t / (r_{ridx(t)} * m_{t-1})
        logm = np.log(mm)                        # (T, BC)
        logr = np.log(np.where(rr > 0, rr, 1.0))  # (T, BC); last rows unused
        ll = logm[0] + np.sum(logm[1:] - logm[:-1] - logr[ridx], axis=0)
        loglik[i * BC : (i + 1) * BC, 0] = ll.astype(np.float32)

    return forward, loglik
